# revision 1
# baseline (speedup 1.0000x reference)
"""EquiMHA Trainium2 kernel.

Data-parallel over batch B=8 across the 8 NeuronCores (one batch element per
core, weights replicated, no collectives).

Per-core computation for batch b (N=512, M=4, KN=512, DQ=DK=512, DV=1024,
H=16, D=64):
  Qp = Q[b] @ w_q, Kp = K[b] @ w_k, Vp = K[b] @ w_v
  E[h,n,k] = sum_{m,d} Qp[n,m,h*64+d] Kp[k,m,h*64+d] / 32
  A = masked_softmax(E)        (max-subtraction skipped: |E|/32 <= ~2, and the
                                max cancels exactly up to the +eps term)
  O[n,m,h*64+d] = sum_k A[h,n,k] Vp[k,m,h*64+d]
  out = O @ w_o

Layout strategy (no big-tensor transposes through DMA — inputs are loaded
naturally and flipped with cheap PE identity-matmul transposes; everything
downstream is produced directly in the layout its consumer wants):
  - P1/P2 produce the projections in a packed transposed layout
    QpP/KpP[h][mp] = [128 = (m in {2mp,2mp+1}, d), n|k 512] so the score
    matmul E^T[k,n] runs at full PE rate with 128-deep contractions.
  - P3 produces Vp2[kc] = [128 k, (h, mp, m%2, d) 4096] so the O matmul
    O^T[(m,d), n] takes its stationary operand as one contiguous slice and
    streams the masked-exp scores EXM[k,n] (512 wide).
  - softmax runs in the [k, n] orientation: exp on ACT, mask multiply on
    Pool, per-column sums via a ones-vector PE matmul, reciprocal on DVE,
    and a rank-1 ones x recip PE matmul to broadcast 1/sum across
    partitions; normalization is fused into the O-psum evacuation.
  - P7 consumes the assembled O^T tiles as stationary operands, so the final
    output comes out in natural [n, dvo] orientation for contiguous stores.

All matmul operands are float32r (tf32-like single-pass PE mode, ~1e-4 rel
error); accumulation is fp32 in PSUM.

SBUF residency: Vp2 (8MB) + QpP (8MB, Q/attention phases only) + mask (1MB)
stay on-chip; KpP and the O^T tiles round-trip through internal DRAM with
one coalesced DMA per head per direction (plus a split read for O^T).
"""

import numpy as np

import concourse.bacc as bacc
import concourse.mybir as mybir
import concourse.tile as tile

F32 = mybir.dt.float32
F32R = mybir.dt.float32r
I32 = mybir.dt.int32
AF = mybir.ActivationFunctionType

B, N, M, KN = 8, 512, 4, 512
DQ, DK, DV, H = 512, 512, 1024, 16
D = DV // H
EPS = 1e-8
SCALE = 1.0 / 32.0  # 1/sqrt(DV)

NC = N // 128    # n chunks
KC = KN // 128   # k chunks
DQC = DQ // 128  # contraction chunks for projections
DVC = DV // 128  # dv chunks (head pairs)


def build_nc():
    nc = bacc.Bacc("TRN2", target_bir_lowering=False, debug=False, num_devices=8)

    q_d = nc.dram_tensor("Q", [N, M, DQ], F32, kind="ExternalInput")
    k_d = nc.dram_tensor("K", [KN, M, DK], F32, kind="ExternalInput")
    mask_d = nc.dram_tensor("mask", [N, KN], I32, kind="ExternalInput")
    wq_d = nc.dram_tensor("w_q", [DQ, DV], F32R, kind="ExternalInput")
    wk_d = nc.dram_tensor("w_k", [DK, DV], F32R, kind="ExternalInput")
    wv_d = nc.dram_tensor("w_v", [DK, DV], F32R, kind="ExternalInput")
    wo_d = nc.dram_tensor("w_o", [DV, DV], F32R, kind="ExternalInput")
    out_d = nc.dram_tensor("out", [N, M, DV], F32, kind="ExternalOutput")

    from concourse.masks import make_identity

    with tile.TileContext(nc) as tc:
        with (
            tc.tile_pool(name="persist", bufs=1) as persist,
            tc.tile_pool(name="dram", bufs=1, space="DRAM") as dram,
        ):
            # --- persistent tensors (whole kernel) ---
            ones_f = persist.tile([128, 128], F32, name="ones_f")
            nc.vector.memset(ones_f, 1.0)
            ones = persist.tile([128, 128], F32R, name="ones")
            nc.vector.tensor_copy(ones, ones_f)
            ident = persist.tile([128, 128], F32, name="ident")
            make_identity(nc, ident)
            ident_r = persist.tile([128, 128], F32R, name="ident_r")
            nc.vector.tensor_copy(ident_r, ident)

            maskTf = [
                persist.tile([128, N], F32, name=f"maskTf_{kc}") for kc in range(KC)
            ]
            vp2 = [
                persist.tile([128, M * DV], F32R, name=f"vp2_{kc}") for kc in range(KC)
            ]
            # KpP round-trip: one [128, 2(mp), 512] record per head
            kpp_d = dram.tile([H, 128, 2, KN], F32R, name="kpp_d")
            # O^T round-trip: per head h: [128 = (m%2, d), mp, n]
            ot_d = dram.tile([H, 128, 2, N], F32R, name="ot_d")

            # =================== K-side projections (P2: KpP, P3: Vp2) =====
            with (
                tc.tile_pool(name="xk", bufs=1) as xk,
                tc.tile_pool(name="wv", bufs=1) as wvp,
                tc.tile_pool(name="wk", bufs=1) as wkp,
                tc.tile_pool(name="kst", bufs=3) as kstp,
            ):
                xkt = [
                    xk.tile([128, M, KN], F32R, name=f"xkt_{c}") for c in range(DQC)
                ]
                wv_sb = [
                    wvp.tile([128, DV], F32R, name=f"wv_{c}") for c in range(DQC)
                ]
                wk_sb = [
                    wkp.tile([128, DV], F32R, name=f"wk_{c}") for c in range(DQC)
                ]

                # natural per-(m-pair) loads + PE transpose into xkt
                with (
                    tc.tile_pool(name="xkn", bufs=6) as xkn,
                    tc.tile_pool(name="pstr", bufs=1, space="PSUM") as pstr,
                ):
                    for mp in range(2):
                        xn = []
                        for r in range(KC):
                            t = xkn.tile(
                                [128, 2, DK], F32R, name=f"xkn_{mp}_{r}", tag="xkn"
                            )
                            nc.sync.dma_start(
                                t,
                                k_d.ap()[
                                    r * 128 : (r + 1) * 128, 2 * mp : 2 * mp + 2, :
                                ].bitcast(F32R),
                            )
                            xn.append(t)
                        for s in range(2):
                            m = 2 * mp + s
                            pts = [
                                pstr.tile(
                                    [128, KN], F32R, name=f"pt{c}", tag=f"pt{c}"
                                )
                                for c in range(DQC)
                            ]
                            for r in range(KC):
                                for c in range(DQC):
                                    nc.tensor.transpose(
                                        pts[c][:, r * 128 : (r + 1) * 128],
                                        xn[r][:, s, c * 128 : (c + 1) * 128],
                                        ident_r,
                                    )
                            for c in range(DQC):
                                nc.scalar.copy(xkt[c][:, m, :], pts[c])

                # weight loads: behind all K-input tiles in the DMA queue
                for c in range(DQC):
                    nc.sync.dma_start(wv_sb[c], wv_d.ap()[c * 128 : (c + 1) * 128])
                    nc.sync.dma_start(wk_sb[c], wk_d.ap()[c * 128 : (c + 1) * 128])

                with tc.tile_pool(name="psproj", bufs=6, space="PSUM") as psproj:
                    # P2: KpP[h] -> DRAM (one staging record per head)
                    for dvc in range(DVC):
                        ka = kstp.tile([128, 2, KN], F32R, name="kstA", tag="kstA")
                        kb = kstp.tile([128, 2, KN], F32R, name="kstB", tag="kstB")
                        for mp in range(2):
                            for s in range(2):
                                m = 2 * mp + s
                                pp = psproj.tile([128, KN], F32, name="pp", tag="pp")
                                for c in range(DQC):
                                    nc.tensor.matmul(
                                        pp,
                                        wk_sb[c][:, dvc * 128 : (dvc + 1) * 128],
                                        xkt[c][:, m, :],
                                        start=(c == 0),
                                        stop=(c == DQC - 1),
                                    )
                                nc.scalar.copy(
                                    ka[s * 64 : (s + 1) * 64, mp, :], pp[0:64, :]
                                )
                                nc.vector.tensor_copy(
                                    kb[s * 64 : (s + 1) * 64, mp, :], pp[64:128, :]
                                )
                        nc.sync.dma_start(kpp_d[2 * dvc], ka)
                        nc.sync.dma_start(kpp_d[2 * dvc + 1], kb)

                    # mask: natural load, int->float convert, PE transpose
                    with (
                        tc.tile_pool(name="mload", bufs=2) as mload,
                        tc.tile_pool(name="psmask", bufs=2, space="PSUM") as psmask,
                    ):
                        mnat = []
                        for ncc in range(NC):
                            mi = mload.tile([128, KN], I32, name=f"mi_{ncc}", tag="mi")
                            nc.sync.dma_start(
                                mi, mask_d.ap()[ncc * 128 : (ncc + 1) * 128]
                            )
                            mf = mload.tile(
                                [128, KN], F32, name=f"mf_{ncc}", tag="mf", bufs=4
                            )
                            nc.vector.tensor_copy(mf, mi)
                            mnat.append(mf)
                        for kc in range(KC):
                            pm = psmask.tile([128, N], F32, name="pm", tag="pm")
                            for ncc in range(NC):
                                nc.tensor.transpose(
                                    pm[:, ncc * 128 : (ncc + 1) * 128],
                                    mnat[ncc][:, kc * 128 : (kc + 1) * 128],
                                    ident,
                                )
                            nc.vector.tensor_copy(maskTf[kc], pm)

                    # P3: Vp2 (SBUF resident)
                    for m in range(M):
                        for kc in range(KC):
                            for dvh in range(2):
                                pp = psproj.tile(
                                    [128, 512], F32, name="pv", tag="pp"
                                )
                                for c in range(DQC):
                                    nc.tensor.matmul(
                                        pp,
                                        xkt[c][:, m, kc * 128 : (kc + 1) * 128],
                                        wv_sb[c][:, dvh * 512 : (dvh + 1) * 512],
                                        start=(c == 0),
                                        stop=(c == DQC - 1),
                                    )
                                # Vp2 column layout: col = h*256 + (m//2)*128
                                # + (m%2)*64 + d -> per-(h,mp) stationary is
                                # one contiguous 128-column run.
                                v4 = vp2[kc].rearrange("p (h c) -> p h c", h=H)
                                off = (m // 2) * 128 + (m % 2) * 64
                                nc.vector.tensor_copy(
                                    v4[:, dvh * 8 : (dvh + 1) * 8, off : off + 64],
                                    pp.rearrange("p (h d) -> p h d", h=8),
                                )

            # first half of w_o, prefetched mid-attention so P7 doesn't
            # stall on its weight load
            wo_a_pool = tc.tile_pool(name="wo_a", bufs=1)
            wopa = wo_a_pool.__enter__()
            wo_a = [wopa.tile([128, DV], F32R, name=f"wo_a{c}") for c in range(3)]

            with tc.tile_pool(name="qpp", bufs=1) as qppp:
                qpp = [
                    [
                        qppp.tile([128, N], F32R, name=f"qpp_{h}_{mp}")
                        for mp in range(2)
                    ]
                    for h in range(H)
                ]

                # =================== Q-side projection (P1: QpP) ===========
                with (
                    tc.tile_pool(name="xq", bufs=1) as xq,
                    tc.tile_pool(name="wqs", bufs=4) as wqs,
                    tc.tile_pool(name="psproj2", bufs=4, space="PSUM") as psproj2,
                ):
                    xqt = [
                        xq.tile([128, M, N], F32R, name=f"xqt_{c}")
                        for c in range(DQC)
                    ]
                    with tc.tile_pool(name="pstr2", bufs=1, space="PSUM") as pstr2:
                        for mp in range(2):
                            xn = []
                            for r in range(NC):
                                t = xq.tile(
                                    [128, 2, DQ],
                                    F32R,
                                    name=f"xqn_{mp}_{r}",
                                    tag="xqn",
                                    bufs=4,
                                )
                                nc.scalar.dma_start(
                                    t,
                                    q_d.ap()[
                                        r * 128 : (r + 1) * 128,
                                        2 * mp : 2 * mp + 2,
                                        :,
                                    ].bitcast(F32R),
                                )
                                xn.append(t)
                            for s in range(2):
                                m = 2 * mp + s
                                pts = [
                                    pstr2.tile(
                                        [128, N], F32R, name=f"pt2{c}", tag=f"pt2{c}"
                                    )
                                    for c in range(DQC)
                                ]
                                for r in range(NC):
                                    for c in range(DQC):
                                        nc.tensor.transpose(
                                            pts[c][:, r * 128 : (r + 1) * 128],
                                            xn[r][:, s, c * 128 : (c + 1) * 128],
                                            ident_r,
                                        )
                                for c in range(DQC):
                                    nc.scalar.copy(xqt[c][:, m, :], pts[c])

                    for dvh in range(2):
                        wqt = [
                            wqs.tile(
                                [128, 512], F32R, name=f"wq_{dvh}_{c}", tag="wq"
                            )
                            for c in range(DQC)
                        ]
                        for c in range(DQC):
                            nc.sync.dma_start(
                                wqt[c],
                                wq_d.ap()[
                                    c * 128 : (c + 1) * 128,
                                    dvh * 512 : (dvh + 1) * 512,
                                ],
                            )
                        for dv4 in range(4):
                            dvc = dvh * 4 + dv4
                            for mp in range(2):
                                for s in range(2):
                                    m = 2 * mp + s
                                    pp = psproj2.tile(
                                        [128, N], F32, name="pq", tag="pq"
                                    )
                                    for c in range(DQC):
                                        nc.tensor.matmul(
                                            pp,
                                            wqt[c][:, dv4 * 128 : (dv4 + 1) * 128],
                                            xqt[c][:, m, :],
                                            start=(c == 0),
                                            stop=(c == DQC - 1),
                                        )
                                    nc.scalar.copy(
                                        qpp[2 * dvc][mp][s * 64 : (s + 1) * 64, :],
                                        pp[0:64, :],
                                    )
                                    nc.vector.tensor_copy(
                                        qpp[2 * dvc + 1][mp][
                                            s * 64 : (s + 1) * 64, :
                                        ],
                                        pp[64:128, :],
                                    )

                # =================== attention, per head ===================
                # Software-pipelined: E/exp/mask for head h+1 are emitted
                # before the sums/normalize/O tail of head h, so the PE
                # streams E(h+1) while head h's softmax runs on ACT/Pool/DVE.
                with (
                    tc.tile_pool(name="kin", bufs=4) as kin,
                    tc.tile_pool(name="expp", bufs=3) as expp,
                    tc.tile_pool(name="exmp", bufs=8) as exmp,
                    tc.tile_pool(name="rp", bufs=2) as rp,
                    tc.tile_pool(name="repp", bufs=1) as repp,
                    tc.tile_pool(name="otst", bufs=2) as otstp,
                    tc.tile_pool(name="pse", bufs=4, space="PSUM") as pse,
                    tc.tile_pool(name="pss", bufs=1, space="PSUM") as pss,
                    tc.tile_pool(name="psr", bufs=1, space="PSUM") as psr,
                    tc.tile_pool(name="pso", bufs=2, space="PSUM") as pso,
                ):

                    def emit_e(h):
                        kt_ = kin.tile([128, 2, KN], F32R, name=f"kin_{h}", tag="kin")
                        nc.scalar.dma_start(kt_, kpp_d[h])
                        exm = []
                        for kc in range(KC):
                            pe = pse.tile([128, N], F32, name="pe", tag="pe")
                            for mp in range(2):
                                nc.tensor.matmul(
                                    pe,
                                    kt_[:, mp, kc * 128 : (kc + 1) * 128],
                                    qpp[h][mp],
                                    start=(mp == 0),
                                    stop=(mp == 1),
                                )
                            ex = expp.tile([128, N], F32, name="ex", tag="ex")
                            nc.scalar.activation(ex, pe, AF.Exp, scale=SCALE)
                            em = exmp.tile([128, N], F32R, name="em", tag="em")
                            nc.gpsimd.tensor_mul(em, ex, maskTf[kc])
                            exm.append(em)
                        return exm

                    def emit_tail(h, exm):
                        ps_ = pss.tile([1, N], F32, name="ps", tag="ps")
                        for kc in range(KC):
                            nc.tensor.matmul(
                                ps_,
                                ones[:, 0:1],
                                exm[kc],
                                start=(kc == 0),
                                stop=(kc == KC - 1),
                            )
                        s_sb = rp.tile([1, N], F32, name="s_sb", tag="s")
                        nc.vector.tensor_scalar_add(s_sb, ps_, EPS)
                        r_sb = rp.tile([1, N], F32R, name="r_sb", tag="r")
                        with nc.allow_low_precision(reason="f32r feeds PE broadcast"):
                            nc.vector.reciprocal(r_sb, s_sb)

                        # O matmuls first; the 1/sum broadcast matmul runs
                        # after them so its wait on the DVE reciprocal is
                        # hidden behind the O streams.
                        pos = []
                        for mp in range(2):
                            po = pso.tile([128, N], F32, name="po", tag="po")
                            for kc in range(KC):
                                nc.tensor.matmul(
                                    po,
                                    vp2[kc][
                                        :,
                                        h * 256
                                        + mp * 128 : h * 256
                                        + (mp + 1) * 128,
                                    ],
                                    exm[kc],
                                    start=(kc == 0),
                                    stop=(kc == KC - 1),
                                )
                            pos.append(po)
                        pr = psr.tile([128, N], F32, name="pr", tag="pr")
                        nc.tensor.matmul(pr, ones[0:1, :], r_sb, start=True, stop=True)
                        rep = repp.tile([128, N], F32, name="rep", tag="rep")
                        nc.vector.tensor_copy(rep, pr)

                        ot = otstp.tile([128, 2, N], F32R, name="ot", tag="ot")
                        for mp in range(2):
                            nc.vector.tensor_mul(ot[:, mp, :], pos[mp], rep)
                        nc.sync.dma_start(ot_d[h], ot)

                    prev_exm = None
                    for h in range(H):
                        cur_exm = emit_e(h)
                        if h == 2:
                            for c in range(3):
                                nc.sync.dma_start(
                                    wo_a[c], wo_d.ap()[c * 128 : (c + 1) * 128]
                                )
                        if prev_exm is not None:
                            emit_tail(h - 1, prev_exm)
                        prev_exm = cur_exm
                    emit_tail(H - 1, prev_exm)

            # =================== output projection (P7) ====================
            with (
                tc.tile_pool(name="wo", bufs=1) as wop,
                tc.tile_pool(name="otin", bufs=14) as otin,
                tc.tile_pool(name="outst", bufs=4) as outstp,
                tc.tile_pool(name="psf", bufs=8, space="PSUM") as psf,
            ):
                wo_sb = list(wo_a)
                wo_rest = []
                for c in range(3, DVC):
                    wt = wop.tile([128, DV], F32R, name=f"wo_{c}")
                    wo_rest.append(wt)
                    wo_sb.append(wt)

                # ot_d[h] holds [(s, d) 128, mp, n]; the P7 stationary tile
                # for (m, dvc) needs rows (h in {2dvc, 2dvc+1}, d) of column
                # block mp = m//2, row half s = m%2.
                otv = ot_d.rearrange("h (s d) t n -> h s d t n", s=2)
                for m in range(M):
                    mp, s = m // 2, m % 2
                    ott = []
                    for dvc in range(DVC):
                        ot = otin.tile(
                            [128, N], F32R, name=f"oti_{m}_{dvc}", tag="oti"
                        )
                        nc.sync.dma_start(
                            ot, otv[2 * dvc : 2 * dvc + 2, s, :, mp, :]
                        )
                        ott.append(ot)
                    if m == 0:
                        # remaining w_o chunks queued behind the first O tiles
                        for c, wt in zip(range(3, DVC), wo_rest):
                            nc.sync.dma_start(
                                wt, wo_d.ap()[c * 128 : (c + 1) * 128]
                            )
                    for ncc in range(NC):
                        ost = outstp.tile([128, 2, 512], F32, name="ost", tag="ost")
                        for dvh in range(2):
                            pf = psf.tile([128, 512], F32, name="pf", tag="pf")
                            for dvc in range(DVC):
                                nc.tensor.matmul(
                                    pf,
                                    ott[dvc][:, ncc * 128 : (ncc + 1) * 128],
                                    wo_sb[dvc][:, dvh * 512 : (dvh + 1) * 512],
                                    start=(dvc == 0),
                                    stop=(dvc == DVC - 1),
                                )
                            nc.scalar.copy(ost[:, dvh, :], pf)
                        nc.sync.dma_start(
                            out_d.ap()[ncc * 128 : (ncc + 1) * 128, m, :],
                            ost.rearrange("p a b -> p (a b)"),
                        )

            wo_a_pool.__exit__(None, None, None)

    nc.compile()
    return nc


_NC_CACHE = None


def _get_nc():
    global _NC_CACHE
    if _NC_CACHE is None:
        _NC_CACHE = build_nc()
    return _NC_CACHE


def kernel(Q, K, mask, w_q, w_k, w_v, w_o):
    from concourse.bass_utils import run_bass_kernel_spmd

    Q = np.ascontiguousarray(np.asarray(Q), dtype=np.float32)
    K = np.ascontiguousarray(np.asarray(K), dtype=np.float32)
    mask = np.ascontiguousarray(np.asarray(mask), dtype=np.int32)
    w_q = np.ascontiguousarray(np.asarray(w_q), dtype=np.float32)
    w_k = np.ascontiguousarray(np.asarray(w_k), dtype=np.float32)
    w_v = np.ascontiguousarray(np.asarray(w_v), dtype=np.float32)
    w_o = np.ascontiguousarray(np.asarray(w_o), dtype=np.float32)

    nc = _get_nc()
    in_maps = [
        {
            "Q": Q[b],
            "K": K[b],
            "mask": mask[b],
            "w_q": w_q,
            "w_k": w_k,
            "w_v": w_v,
            "w_o": w_o,
        }
        for b in range(B)
    ]
    r = run_bass_kernel_spmd(nc, in_maps, core_ids=list(range(B)), trace=False)
    return np.stack([r.results[b]["out"] for b in range(B)], axis=0)


if __name__ == "__main__":
    rng = np.random.default_rng(0)
    inputs = {
        "Q": rng.standard_normal((B, N, M, DQ), dtype=np.float32),
        "K": rng.standard_normal((B, KN, M, DK), dtype=np.float32),
        "mask": rng.integers(0, 2, (B, N, KN)).astype(np.int32),
        "w_q": (rng.standard_normal((DQ, DV), dtype=np.float32) * 0.02),
        "w_k": (rng.standard_normal((DK, DV), dtype=np.float32) * 0.02),
        "w_v": (rng.standard_normal((DK, DV), dtype=np.float32) * 0.02),
        "w_o": (rng.standard_normal((DV, DV), dtype=np.float32) * 0.02),
    }
    out = kernel(**inputs)
    print("out", out.shape, out.dtype, float(np.abs(out).max()))



# revision 4
# speedup vs baseline: 1.0759x; 1.0759x over previous
"""EquiMHA Trainium2 kernel.

Data-parallel over batch B=8 across the 8 NeuronCores (one batch element per
core, weights replicated, no collectives).

Per-core computation for batch b (N=512, M=4, KN=512, DQ=DK=512, DV=1024,
H=16, D=64):
  Qp = Q[b] @ w_q, Kp = K[b] @ w_k, Vp = K[b] @ w_v
  E[h,n,k] = sum_{m,d} Qp[n,m,h*64+d] Kp[k,m,h*64+d] / 32
  A = masked_softmax(E)        (max-subtraction skipped: |E|/32 <= ~2, and the
                                max cancels exactly up to the +eps term)
  O[n,m,h*64+d] = sum_k A[h,n,k] Vp[k,m,h*64+d]
  out = O @ w_o

Precision strategy (tolerance is 2e-2; measured pipeline error ~1e-2):
  - Q/K-side projections run in fp8e4m3 with DoubleRow perf mode (2x PE
    rate, 256-deep contraction per pass). Host pre-quantizes Q, K and
    w_q, w_k (weights pre-scaled by 64 into fp8 normal range; the 64*64
    factor is folded into the exp scale).
  - Everything downstream (E scores, A@V, O@w_o) runs in bf16, which is
    full PE rate and halves SBUF/DMA vs f32 so the whole working set
    stays on-chip (no DRAM round trips).

Layout strategy: the host pre-transposes all inputs into the exact tile
layouts the PE wants, so the kernel does zero on-chip transposes:
  - QT8/KT8 [256,2,M,N]: [c*128+p, i, m, n] with dq = c*256+i*128+p, the
    DoubleRow pairing of two 128-deep contraction chunks.
  - P1/P2 emit qpp/kpp[h] = [(s,d) 128, mp, n|k] bf16 via partition-shifted
    psum evacuation (DoubleRow outputs land on psum partitions 0-63).
  - E^T[k,n] per head is a 2-matmul bf16 contraction over mp; softmax runs
    in [k, n] orientation: exp on ACT (bf16 out), mask multiply on Pool,
    per-column sums via a bf16 ones-vector PE matmul, reciprocal on DVE,
    and a gpsimd partition_broadcast for 1/sum (no PE broadcast matmul).
  - Vp2[kc] = [128 k, (h, mp, s, d) 4096] bf16 so the O matmul stationary
    is one contiguous 128-column slice; O psum is normalized on DVE
    directly into OT[hp] = [(h%2,d), m, n] bf16 tiles, which are exactly
    the stationary operands P7 wants. Output leaves in natural [n, m, dvo]
    orientation for contiguous stores.
"""

import numpy as np
import ml_dtypes

import concourse.bacc as bacc
import concourse.mybir as mybir
import concourse.tile as tile

F32 = mybir.dt.float32
F32R = mybir.dt.float32r
F8 = mybir.dt.float8e4
BF = mybir.dt.bfloat16
AF = mybir.ActivationFunctionType
DR = mybir.MatmulPerfMode.DoubleRow

NPF8 = ml_dtypes.float8_e4m3
NPBF = ml_dtypes.bfloat16

B, N, M, KN = 8, 512, 4, 512
DQ, DK, DV, H = 512, 512, 1024, 16
D = DV // H
HP = H // 2          # head pairs (P7 contraction chunks)
KC = KN // 128       # k chunks
NC = N // 128        # n chunks
WS = 64.0            # host pre-scale for fp8 weights
SCALE = 1.0 / 32.0   # 1/sqrt(DV)
ESCALE = SCALE / (WS * WS)  # fused into exp


def build_nc():
    nc = bacc.Bacc("TRN2", target_bir_lowering=False, debug=False, num_devices=8)

    qt8_d = nc.dram_tensor("QT8", [256, 2, M, N], F8, kind="ExternalInput")
    kt8_d = nc.dram_tensor("KT8", [256, 2, M, KN], F8, kind="ExternalInput")
    ktb_d = nc.dram_tensor("KTB", [DK, M, KN], BF, kind="ExternalInput")
    mt_d = nc.dram_tensor("MT", [KN, N], BF, kind="ExternalInput")
    wq8_d = nc.dram_tensor("WQ8", [256, 2, DV], F8, kind="ExternalInput")
    wk8_d = nc.dram_tensor("WK8", [256, 2, DV], F8, kind="ExternalInput")
    wvb_d = nc.dram_tensor("WVB", [DK, DV], BF, kind="ExternalInput")
    wob_d = nc.dram_tensor("WOB", [DV, DV], BF, kind="ExternalInput")
    out_d = nc.dram_tensor("out", [N, M, DV], F32, kind="ExternalOutput")

    def ecopy(eng, dst, src):
        if eng is nc.scalar:
            nc.scalar.copy(dst, src)
        else:
            eng.tensor_copy(dst, src)

    with tile.TileContext(nc) as tc:
        with tc.tile_pool(name="persist", bufs=1) as persist:
            onesf = persist.tile([128, 128], F32, name="onesf")
            nc.vector.memset(onesf, 1.0)
            onesb = persist.tile([128, 1], BF, name="onesb")
            nc.vector.tensor_copy(onesb, onesf[:, 0:1])

            maskT = [persist.tile([128, N], BF, name=f"mT{kc}") for kc in range(KC)]
            vp2 = [persist.tile([128, M * DV], BF, name=f"vp2_{kc}") for kc in range(KC)]
            qpp = [persist.tile([128, 2, N], BF, name=f"qpp{h}") for h in range(H)]
            kpp = [persist.tile([128, 2, KN], BF, name=f"kpp{h}") for h in range(H)]
            ot = [persist.tile([128, M, N], BF, name=f"ot{hp}") for hp in range(HP)]
            wob = [persist.tile([128, DV], BF, name=f"wob{hp}") for hp in range(HP)]

            # ============ P2: Kp projection (fp8 DoubleRow) ============
            with (
                tc.tile_pool(name="xk8", bufs=1) as xk8p,
                tc.tile_pool(name="wk8", bufs=1) as wk8p,
                tc.tile_pool(name="ppk", bufs=3, space="PSUM") as ppk,
            ):
                xk8 = [xk8p.tile([128, 2, M, KN], F8, name=f"xk8_{c}") for c in range(2)]
                wk8 = [wk8p.tile([128, 2, DV], F8, name=f"wk8_{c}") for c in range(2)]
                for c in range(2):
                    nc.sync.dma_start(xk8[c], kt8_d.ap()[c * 128 : (c + 1) * 128])
                    nc.sync.dma_start(wk8[c], wk8_d.ap()[c * 128 : (c + 1) * 128])

                def proj8(h, s, w_sb, x_sb, dst, eng):
                    pq2 = ppk.tile([64, 1024], F32, name="pq2", tag="pq2")
                    for mp in range(2):
                        m = 2 * mp + s
                        for nh in range(2):
                            for c in range(2):
                                nc.tensor.matmul(
                                    pq2[:, mp * 512 + nh * 256 : mp * 512 + (nh + 1) * 256],
                                    w_sb[c][:, :, h * 64 : (h + 1) * 64],
                                    x_sb[c][:, :, m, nh * 256 : (nh + 1) * 256],
                                    start=(c == 0),
                                    stop=(c == 1),
                                    perf_mode=DR,
                                )
                    ecopy(
                        eng,
                        dst[s * 64 : (s + 1) * 64, :, :],
                        pq2.rearrange("p (t n) -> p t n", t=2),
                    )

                for h in range(H):
                    for s in range(2):
                        eng = nc.scalar if s == 0 else nc.vector
                        proj8(h, s, wk8, xk8, kpp[h], eng)

                # mask + wob loads ride behind the K-side loads
                for kc in range(KC):
                    nc.sync.dma_start(maskT[kc], mt_d.ap()[kc * 128 : (kc + 1) * 128])
                for hp in range(HP):
                    nc.sync.dma_start(wob[hp], wob_d.ap()[hp * 128 : (hp + 1) * 128])

            # ============ P3: Vp projection (bf16) ============
            with (
                tc.tile_pool(name="xkb", bufs=1) as xkbp,
                tc.tile_pool(name="wvb", bufs=1) as wvbp,
                tc.tile_pool(name="ppv", bufs=2, space="PSUM") as ppv,
            ):
                xkb = [xkbp.tile([128, M, KN], BF, name=f"xkb{c}") for c in range(4)]
                wvb = [wvbp.tile([128, DV], BF, name=f"wvb{c}") for c in range(4)]
                for c in range(4):
                    nc.sync.dma_start(xkb[c], ktb_d.ap()[c * 128 : (c + 1) * 128])
                    nc.sync.dma_start(wvb[c], wvb_d.ap()[c * 128 : (c + 1) * 128])

                for mi in range(M):
                    off = (mi // 2) * 128 + (mi % 2) * 64
                    for kc in range(KC):
                        for dvh in range(2):
                            pv = ppv.tile([128, 512], F32, name="pv", tag="pv")
                            for c in range(4):
                                nc.tensor.matmul(
                                    pv,
                                    xkb[c][:, mi, kc * 128 : (kc + 1) * 128],
                                    wvb[c][:, dvh * 512 : (dvh + 1) * 512],
                                    start=(c == 0),
                                    stop=(c == 3),
                                )
                            v4 = vp2[kc].rearrange("p (h c) -> p h c", h=H)
                            eng = nc.scalar if (kc + dvh) % 2 == 0 else nc.vector
                            ecopy(
                                eng,
                                v4[:, dvh * 8 : (dvh + 1) * 8, off : off + 64],
                                pv.rearrange("p (h d) -> p h d", h=8),
                            )

            # ============ P1: Qp projection (fp8 DoubleRow) ============
            with (
                tc.tile_pool(name="xq8", bufs=1) as xq8p,
                tc.tile_pool(name="wq8", bufs=1) as wq8p,
                tc.tile_pool(name="ppq", bufs=3, space="PSUM") as ppq,
            ):
                xq8 = [xq8p.tile([128, 2, M, N], F8, name=f"xq8_{c}") for c in range(2)]
                wq8 = [wq8p.tile([128, 2, DV], F8, name=f"wq8_{c}") for c in range(2)]
                for c in range(2):
                    nc.sync.dma_start(xq8[c], qt8_d.ap()[c * 128 : (c + 1) * 128])
                    nc.sync.dma_start(wq8[c], wq8_d.ap()[c * 128 : (c + 1) * 128])

                def proj8q(h, s, eng):
                    pq2 = ppq.tile([64, 1024], F32, name="pq2q", tag="pq2q")
                    for mp in range(2):
                        m = 2 * mp + s
                        for nh in range(2):
                            for c in range(2):
                                nc.tensor.matmul(
                                    pq2[:, mp * 512 + nh * 256 : mp * 512 + (nh + 1) * 256],
                                    wq8[c][:, :, h * 64 : (h + 1) * 64],
                                    xq8[c][:, :, m, nh * 256 : (nh + 1) * 256],
                                    start=(c == 0),
                                    stop=(c == 1),
                                    perf_mode=DR,
                                )
                    ecopy(
                        eng,
                        qpp[h][s * 64 : (s + 1) * 64, :, :],
                        pq2.rearrange("p (t n) -> p t n", t=2),
                    )

                for h in range(H):
                    for s in range(2):
                        eng = nc.scalar if s == 0 else nc.vector
                        proj8q(h, s, eng)

            # ============ attention, per head (software-pipelined) ============
            with (
                tc.tile_pool(name="exp", bufs=4) as expp,
                tc.tile_pool(name="emp", bufs=8) as emp,
                tc.tile_pool(name="rp", bufs=2) as rp,
                tc.tile_pool(name="repp", bufs=2) as repp,
                tc.tile_pool(name="pse", bufs=3, space="PSUM") as pse,
                tc.tile_pool(name="pss", bufs=1, space="PSUM") as pss,
                tc.tile_pool(name="pso", bufs=4, space="PSUM") as pso,
            ):

                def emit_e(h):
                    em = []
                    for kc in range(KC):
                        pe = pse.tile([128, N], F32, name="pe", tag="pe")
                        for mp in range(2):
                            nc.tensor.matmul(
                                pe,
                                kpp[h][:, mp, kc * 128 : (kc + 1) * 128],
                                qpp[h][:, mp, :],
                                start=(mp == 0),
                                stop=(mp == 1),
                            )
                        ex = expp.tile([128, N], BF, name="ex", tag="ex")
                        nc.scalar.activation(ex, pe, AF.Exp, scale=ESCALE)
                        emt = emp.tile([128, N], BF, name="em", tag="em")
                        nc.gpsimd.tensor_mul(emt, ex, maskT[kc])
                        em.append(emt)
                    return em

                def emit_tail(h, em):
                    ps_ = pss.tile([1, N], F32, name="ps", tag="ps")
                    for kc in range(KC):
                        nc.tensor.matmul(
                            ps_,
                            onesb,
                            em[kc],
                            start=(kc == 0),
                            stop=(kc == KC - 1),
                        )
                    r_sb = rp.tile([1, N], F32, name="r_sb", tag="r")
                    with nc.allow_low_precision(reason="softmax 1/sum"):
                        nc.vector.reciprocal(r_sb, ps_)

                    pos = []
                    for mp in range(2):
                        po = pso.tile([128, N], F32, name="po", tag="po")
                        for kc in range(KC):
                            nc.tensor.matmul(
                                po,
                                vp2[kc][:, h * 256 + mp * 128 : h * 256 + (mp + 1) * 128],
                                em[kc],
                                start=(kc == 0),
                                stop=(kc == KC - 1),
                            )
                        pos.append(po)

                    rep = repp.tile([128, N], F32, name="rep", tag="rep")
                    nc.gpsimd.partition_broadcast(rep, r_sb, channels=128)
                    hp, hs = h // 2, h % 2
                    for mp in range(2):
                        for s in range(2):
                            nc.vector.tensor_mul(
                                ot[hp][hs * 64 : (hs + 1) * 64, 2 * mp + s, :],
                                pos[mp][s * 64 : (s + 1) * 64, :],
                                rep[s * 64 : (s + 1) * 64, :],
                            )

                prev = None
                for h in range(H):
                    cur = emit_e(h)
                    if prev is not None:
                        emit_tail(h - 1, prev)
                    prev = cur
                emit_tail(H - 1, prev)

            # ============ P7: output projection (bf16) ============
            with (
                tc.tile_pool(name="outst", bufs=4) as outstp,
                tc.tile_pool(name="psf", bufs=4, space="PSUM") as psf,
            ):
                for mi in range(M):
                    for ncc in range(NC):
                        ost = outstp.tile([128, 2, 512], F32, name="ost", tag="ost")
                        for dvh in range(2):
                            pf = psf.tile([128, 512], F32, name="pf", tag="pf")
                            for hp in range(HP):
                                nc.tensor.matmul(
                                    pf,
                                    ot[hp][:, mi, ncc * 128 : (ncc + 1) * 128],
                                    wob[hp][:, dvh * 512 : (dvh + 1) * 512],
                                    start=(hp == 0),
                                    stop=(hp == HP - 1),
                                )
                            nc.scalar.copy(ost[:, dvh, :], pf)
                        nc.sync.dma_start(
                            out_d.ap()[ncc * 128 : (ncc + 1) * 128, mi, :],
                            ost.rearrange("p a b -> p (a b)"),
                        )

    nc.compile()
    return nc


_NC_CACHE = None


def _get_nc():
    global _NC_CACHE
    if _NC_CACHE is None:
        _NC_CACHE = build_nc()
    return _NC_CACHE


def _dr_pack(a):
    """[512, ...] rows dq = c*256 + i*128 + p -> [256 = c*128+p, 2 = i, ...]"""
    s = a.shape[1:]
    return a.reshape(2, 2, 128, *s).transpose(0, 2, 1, *range(3, 3 + len(s))).reshape(
        256, 2, *s
    )


def kernel(Q, K, mask, w_q, w_k, w_v, w_o):
    from concourse.bass_utils import run_bass_kernel_spmd

    Q = np.asarray(Q, dtype=np.float32)
    K = np.asarray(K, dtype=np.float32)
    mask = np.asarray(mask)
    w_q = np.asarray(w_q, dtype=np.float32)
    w_k = np.asarray(w_k, dtype=np.float32)
    w_v = np.asarray(w_v, dtype=np.float32)
    w_o = np.asarray(w_o, dtype=np.float32)

    wq8 = np.ascontiguousarray(_dr_pack(w_q * WS)).astype(NPF8)
    wk8 = np.ascontiguousarray(_dr_pack(w_k * WS)).astype(NPF8)
    wvb = w_v.astype(NPBF)
    wob = w_o.astype(NPBF)

    in_maps = []
    for b in range(B):
        qt = np.ascontiguousarray(Q[b].transpose(2, 1, 0))   # [DQ, M, N]
        kt = np.ascontiguousarray(K[b].transpose(2, 1, 0))   # [DK, M, KN]
        in_maps.append(
            {
                "QT8": np.ascontiguousarray(_dr_pack(qt)).astype(NPF8),
                "KT8": np.ascontiguousarray(_dr_pack(kt)).astype(NPF8),
                "KTB": kt.astype(NPBF),
                "MT": np.ascontiguousarray(mask[b].T).astype(NPBF),
                "WQ8": wq8,
                "WK8": wk8,
                "WVB": wvb,
                "WOB": wob,
            }
        )

    nc = _get_nc()
    r = run_bass_kernel_spmd(nc, in_maps, core_ids=list(range(B)), trace=False)
    return np.stack([r.results[b]["out"] for b in range(B)], axis=0)


if __name__ == "__main__":
    rng = np.random.default_rng(0)
    inputs = {
        "Q": rng.standard_normal((B, N, M, DQ), dtype=np.float32),
        "K": rng.standard_normal((B, KN, M, DK), dtype=np.float32),
        "mask": rng.integers(0, 2, (B, N, KN)).astype(np.int32),
        "w_q": (rng.standard_normal((DQ, DV), dtype=np.float32) * 0.02),
        "w_k": (rng.standard_normal((DK, DV), dtype=np.float32) * 0.02),
        "w_v": (rng.standard_normal((DK, DV), dtype=np.float32) * 0.02),
        "w_o": (rng.standard_normal((DV, DV), dtype=np.float32) * 0.02),
    }
    out = kernel(**inputs)
    print("out", out.shape, out.dtype, float(np.abs(out).max()))


# revision 11
# speedup vs baseline: 1.3640x; 1.2678x over previous
"""EquiMHA Trainium2 kernel.

Data-parallel over batch B=8 across the 8 NeuronCores (one batch element per
core, weights replicated, no collectives).

Per-core computation for batch b (N=512, M=4, KN=512, DQ=DK=512, DV=1024,
H=16, D=64):
  Qp = Q[b] @ w_q, Kp = K[b] @ w_k, Vp = K[b] @ w_v
  E[h,n,k] = sum_{m,d} Qp[n,m,h*64+d] Kp[k,m,h*64+d] / 32
  A = masked_softmax(E)        (max-subtraction skipped: |E|/32 <= ~2, and the
                                max cancels exactly up to the +eps term)
  O[n,m,h*64+d] = sum_k A[h,n,k] Vp[k,m,h*64+d]
  out = O @ w_o

Precision strategy (tolerance is 2e-2; measured pipeline error ~1e-2):
  - Q/K-side projections run in fp8e4m3 with DoubleRow perf mode (2x PE
    rate, 256-deep contraction per pass). Host pre-quantizes Q, K and
    w_q, w_k (weights pre-scaled by 64 into fp8 normal range; the 64*64
    factor is folded into the exp scale).
  - Everything downstream (E scores, A@V, O@w_o) runs in bf16, which is
    full PE rate and halves SBUF/DMA vs f32 so the whole working set
    stays on-chip (no DRAM round trips).

Layout strategy: the host pre-transposes all inputs into the exact tile
layouts the PE wants, so the kernel does zero on-chip transposes:
  - QT8/KT8 [256,2,M,N]: [c*128+p, i, m, n] with dq = c*256+i*128+p, the
    DoubleRow pairing of two 128-deep contraction chunks.
  - P1/P2 emit qpp/kpp[h] = [(s,d) 128, mp, n|k] bf16 via partition-shifted
    psum evacuation (DoubleRow outputs land on psum partitions 0-63).
  - E^T[k,n] per head is a 2-matmul bf16 contraction over mp; softmax runs
    in [k, n] orientation: exp on ACT (bf16 out), mask multiply on Pool,
    per-column sums via a bf16 ones-vector PE matmul, reciprocal on DVE,
    and a gpsimd partition_broadcast for 1/sum (no PE broadcast matmul).
  - Vp2[kc] = [128 k, (h, mp, s, d) 4096] bf16 so the O matmul stationary
    is one contiguous 128-column slice; O psum is normalized on DVE
    directly into OT[hp] = [(h%2,d), m, n] bf16 tiles, which are exactly
    the stationary operands P7 wants. Output leaves in natural [n, m, dvo]
    orientation for contiguous stores.
"""

import numpy as np
import ml_dtypes

import concourse.bacc as bacc
import concourse.mybir as mybir
import concourse.tile as tile

F32 = mybir.dt.float32
F32R = mybir.dt.float32r
F8 = mybir.dt.float8e4
BF = mybir.dt.bfloat16
AF = mybir.ActivationFunctionType
DR = mybir.MatmulPerfMode.DoubleRow

NPF8 = ml_dtypes.float8_e4m3
NPBF = ml_dtypes.bfloat16

B, N, M, KN = 8, 512, 4, 512
DQ, DK, DV, H = 512, 512, 1024, 16
D = DV // H
HP = H // 2          # head pairs (P7 contraction chunks)
KC = KN // 128       # k chunks
NC = N // 128        # n chunks
WS = 64.0            # host pre-scale for fp8 weights
SCALE = 1.0 / 32.0   # 1/sqrt(DV)
ESCALE = SCALE / (WS * WS)  # fused into exp


def build_nc():
    nc = bacc.Bacc("TRN2", target_bir_lowering=False, debug=False, num_devices=8)

    qt8_d = nc.dram_tensor("QT8", [256, 2, M, N], F8, kind="ExternalInput")
    kt8_d = nc.dram_tensor("KT8", [256, 2, M, KN], F8, kind="ExternalInput")
    ktb_d = nc.dram_tensor("KTB", [DK, M, KN], BF, kind="ExternalInput")
    mt_d = nc.dram_tensor("MT", [KN, N], BF, kind="ExternalInput")
    wq8_d = nc.dram_tensor("WQ8", [256, 2, DV], F8, kind="ExternalInput")
    wk8_d = nc.dram_tensor("WK8", [256, 2, DV], F8, kind="ExternalInput")
    wvb_d = nc.dram_tensor("WVB", [DK, DV], BF, kind="ExternalInput")
    wob_d = nc.dram_tensor("WOB", [DV, DV], BF, kind="ExternalInput")
    out_d = nc.dram_tensor("out", [N, M, DV], F32, kind="ExternalOutput")

    def ecopy(eng, dst, src):
        if eng is nc.scalar:
            nc.scalar.copy(dst, src)
        else:
            eng.tensor_copy(dst, src)

    with tile.TileContext(nc) as tc:
        with tc.tile_pool(name="persist", bufs=1) as persist:
            onesf = persist.tile([128, 128], F32, name="onesf")
            nc.vector.memset(onesf, 1.0)
            onesb = persist.tile([128, 1], BF, name="onesb")
            nc.vector.tensor_copy(onesb, onesf[:, 0:1])

            maskT = [persist.tile([128, N], BF, name=f"mT{kc}") for kc in range(KC)]
            vp2 = [persist.tile([128, M * DV], BF, name=f"vp2_{kc}") for kc in range(KC)]
            qpp = [persist.tile([128, 2, N], BF, name=f"qpp{h}") for h in range(H)]
            kpp = [persist.tile([128, 2, KN], BF, name=f"kpp{h}") for h in range(H)]
            ot = [persist.tile([128, M, N], BF, name=f"ot{hp}") for hp in range(HP)]
            wob = [persist.tile([128, DV], BF, name=f"wob{hp}") for hp in range(HP)]

            # ============ projections ============
            # Phase A: P2 (fp8 DR, evac-heavy) interleaved with half of P3
            # (bf16, PE-heavy); phase B: P1 interleaved with the other half.
            # Interleaving keeps the psum-evac engines (ACT/DVE) loaded
            # continuously instead of bursting past PE per phase.
            eng_ns = {"act": 0.0, "dve": 0.0}

            def pick_eng(act_cost, dve_cost):
                if eng_ns["act"] + act_cost <= eng_ns["dve"] + dve_cost:
                    eng_ns["act"] += act_cost
                    return nc.scalar
                eng_ns["dve"] += dve_cost
                return nc.vector

            with (
                tc.tile_pool(name="xk8", bufs=1) as xk8p,
                tc.tile_pool(name="w8", bufs=1) as w8p,
                tc.tile_pool(name="xkb", bufs=1) as xkbp,
                tc.tile_pool(name="wvb", bufs=1) as wvbp,
                tc.tile_pool(name="ppj", bufs=3, space="PSUM") as ppj,
                tc.tile_pool(name="ppv", bufs=2, space="PSUM") as ppv,
            ):
                xk8 = [xk8p.tile([128, 2, M, KN], F8, name=f"xk8_{c}") for c in range(2)]
                wk8 = [w8p.tile([128, 2, DV], F8, name=f"wk8_{c}") for c in range(2)]
                xkb = [xkbp.tile([128, M, KN], BF, name=f"xkb{c}") for c in range(4)]
                wvb = [wvbp.tile([128, DV], BF, name=f"wvb{c}") for c in range(4)]
                xq8 = [xk8p.tile([128, 2, M, N], F8, name=f"xq8_{c}") for c in range(2)]
                wq8 = [w8p.tile([128, 2, DV], F8, name=f"wq8_{c}") for c in range(2)]
                for c in range(2):
                    nc.sync.dma_start(wk8[c], wk8_d.ap()[c * 128 : (c + 1) * 128])
                    nc.sync.dma_start(xk8[c], kt8_d.ap()[c * 128 : (c + 1) * 128])
                for c in range(4):
                    nc.sync.dma_start(wvb[c], wvb_d.ap()[c * 128 : (c + 1) * 128])
                    nc.sync.dma_start(xkb[c], ktb_d.ap()[c * 128 : (c + 1) * 128])
                for c in range(2):
                    nc.sync.dma_start(wq8[c], wq8_d.ap()[c * 128 : (c + 1) * 128])
                    nc.sync.dma_start(xq8[c], qt8_d.ap()[c * 128 : (c + 1) * 128])
                for kc in range(KC):
                    nc.sync.dma_start(maskT[kc], mt_d.ap()[kc * 128 : (kc + 1) * 128])
                for hp in range(HP):
                    nc.sync.dma_start(wob[hp], wob_d.ap()[hp * 128 : (hp + 1) * 128])

                def proj8(h, s, w_sb, x_sb, dst):
                    pq2 = ppj.tile([64, 1024], F32, name="pq2", tag="pq2")
                    for mp in range(2):
                        m = 2 * mp + s
                        for nh in range(2):
                            for c in range(2):
                                nc.tensor.matmul(
                                    pq2[:, mp * 512 + nh * 256 : mp * 512 + (nh + 1) * 256],
                                    w_sb[c][:, :, h * 64 : (h + 1) * 64],
                                    x_sb[c][:, :, m, nh * 256 : (nh + 1) * 256],
                                    start=(c == 0),
                                    stop=(c == 1),
                                    perf_mode=DR,
                                )
                    ecopy(
                        pick_eng(1030, 1310),
                        dst[s * 64 : (s + 1) * 64, :, :],
                        pq2.rearrange("p (t n) -> p t n", t=2),
                    )

                p3_units = [
                    (mi, kc, dvh)
                    for mi in range(M)
                    for kc in range(KC)
                    for dvh in range(2)
                ]

                def p3_unit(u):
                    mi, kc, dvh = u
                    off = (mi // 2) * 128 + (mi % 2) * 64
                    pv = ppv.tile([128, 512], F32, name="pv", tag="pv")
                    for c in range(4):
                        nc.tensor.matmul(
                            pv,
                            xkb[c][:, mi, kc * 128 : (kc + 1) * 128],
                            wvb[c][:, dvh * 512 : (dvh + 1) * 512],
                            start=(c == 0),
                            stop=(c == 3),
                        )
                    v4 = vp2[kc].rearrange("p (h c) -> p h c", h=H)
                    ecopy(
                        pick_eng(610, 730),
                        v4[:, dvh * 8 : (dvh + 1) * 8, off : off + 64],
                        pv.rearrange("p (h d) -> p h d", h=8),
                    )

                p3i = 0
                # phase A: P2 x16 heads; two P3 units per head from h=8 on
                for h in range(H):
                    proj8(h, 0, wk8, xk8, kpp[h])
                    proj8(h, 1, wk8, xk8, kpp[h])
                    if h >= 8:
                        p3_unit(p3_units[p3i]); p3i += 1
                        p3_unit(p3_units[p3i]); p3i += 1
                # phase B: P1 x16 heads; one P3 unit per head
                for h in range(H):
                    proj8(h, 0, wq8, xq8, qpp[h])
                    proj8(h, 1, wq8, xq8, qpp[h])
                    if p3i < len(p3_units):
                        p3_unit(p3_units[p3i]); p3i += 1
                while p3i < len(p3_units):
                    p3_unit(p3_units[p3i]); p3i += 1

            # ============ attention, per head (software-pipelined) ============
            # Iteration h emits: E(h) -> norm(h-2) -> sums/O/bcast(h-1).
            # Normalization lags two heads so the DVE muls never wait on the
            # reciprocal/broadcast chain; mask-muls and normalize run on DVE
            # in bf16 (4x fast mode), O-psum evacuates through ACT.
            with (
                tc.tile_pool(name="exp", bufs=4) as expp,
                tc.tile_pool(name="emp", bufs=8) as emp,
                tc.tile_pool(name="rp", bufs=2) as rp,
                tc.tile_pool(name="repp", bufs=2) as repp,
                tc.tile_pool(name="opop", bufs=3) as opop,
                tc.tile_pool(name="pse", bufs=4, space="PSUM") as pse,
                tc.tile_pool(name="pss", bufs=1, space="PSUM") as pss,
                tc.tile_pool(name="pso", bufs=3, space="PSUM") as pso,  # 4+1+3 = 8
            ):

                def emit_e(h):
                    em = []
                    for kc in range(KC):
                        pe = pse.tile([128, N], F32, name="pe", tag="pe")
                        for mp in range(2):
                            nc.tensor.matmul(
                                pe,
                                kpp[h][:, mp, kc * 128 : (kc + 1) * 128],
                                qpp[h][:, mp, :],
                                start=(mp == 0),
                                stop=(mp == 1),
                            )
                        ex = expp.tile([128, N], BF, name="ex", tag="ex")
                        nc.scalar.activation(ex, pe, AF.Exp, scale=ESCALE)
                        emt = emp.tile([128, N], BF, name="em", tag="em")
                        nc.vector.tensor_mul(emt, ex, maskT[kc])
                        em.append(emt)
                    return em

                def emit_so(h, em):
                    """sums + reciprocal + O matmuls + O evac + 1/sum bcast"""
                    ps_ = pss.tile([1, N], F32, name="ps", tag="ps")
                    for kc in range(KC):
                        nc.tensor.matmul(
                            ps_,
                            onesb,
                            em[kc],
                            start=(kc == 0),
                            stop=(kc == KC - 1),
                        )
                    r_sb = rp.tile([1, N], BF, name="r_sb", tag="r")
                    with nc.allow_low_precision(reason="softmax 1/sum"):
                        nc.vector.reciprocal(r_sb, ps_)

                    opo = opop.tile([128, 2, N], BF, name="opo", tag="opo")
                    for mp in range(2):
                        po = pso.tile([128, N], F32, name="po", tag="po")
                        for kc in range(KC):
                            nc.tensor.matmul(
                                po,
                                vp2[kc][:, h * 256 + mp * 128 : h * 256 + (mp + 1) * 128],
                                em[kc],
                                start=(kc == 0),
                                stop=(kc == KC - 1),
                            )
                        nc.scalar.copy(opo[:, mp, :], po)

                    rep = repp.tile([128, N], BF, name="rep", tag="rep")
                    nc.gpsimd.partition_broadcast(rep, r_sb, channels=128)
                    return opo, rep

                def emit_norm(h, opo, rep):
                    """normalize O into OT tiles (bf16 DVE fast muls)"""
                    hp, hs = h // 2, h % 2
                    for mp in range(2):
                        for s in range(2):
                            nc.vector.tensor_mul(
                                ot[hp][hs * 64 : (hs + 1) * 64, 2 * mp + s, :],
                                opo[s * 64 : (s + 1) * 64, mp, :],
                                rep[s * 64 : (s + 1) * 64, :],
                            )

                em_q, so_q = {}, {}
                for h in range(H):
                    em_q[h] = emit_e(h)
                    if h >= 2:
                        emit_norm(h - 2, *so_q.pop(h - 2))
                    if h >= 1:
                        so_q[h - 1] = emit_so(h - 1, em_q.pop(h - 1))
                so_q[H - 1] = emit_so(H - 1, em_q.pop(H - 1))
                emit_norm(H - 2, *so_q.pop(H - 2))
                emit_norm(H - 1, *so_q.pop(H - 1))

            # ============ P7: output projection (bf16) ============
            with (
                tc.tile_pool(name="outst", bufs=4) as outstp,
                tc.tile_pool(name="psf", bufs=4, space="PSUM") as psf,
            ):
                for mi in range(M):
                    for ncc in range(NC):
                        ost = outstp.tile([128, 2, 512], F32, name="ost", tag="ost")
                        for dvh in range(2):
                            pf = psf.tile([128, 512], F32, name="pf", tag="pf")
                            for hp in range(HP):
                                nc.tensor.matmul(
                                    pf,
                                    ot[hp][:, mi, ncc * 128 : (ncc + 1) * 128],
                                    wob[hp][:, dvh * 512 : (dvh + 1) * 512],
                                    start=(hp == 0),
                                    stop=(hp == HP - 1),
                                )
                            nc.scalar.copy(ost[:, dvh, :], pf)
                        nc.sync.dma_start(
                            out_d.ap()[ncc * 128 : (ncc + 1) * 128, mi, :],
                            ost.rearrange("p a b -> p (a b)"),
                        )

    nc.compile()
    return nc


_NC_CACHE = None


def _get_nc():
    global _NC_CACHE
    if _NC_CACHE is None:
        _NC_CACHE = build_nc()
    return _NC_CACHE


def _dr_pack(a):
    """[512, ...] rows dq = c*256 + i*128 + p -> [256 = c*128+p, 2 = i, ...]"""
    s = a.shape[1:]
    return a.reshape(2, 2, 128, *s).transpose(0, 2, 1, *range(3, 3 + len(s))).reshape(
        256, 2, *s
    )


def kernel(Q, K, mask, w_q, w_k, w_v, w_o):
    from concourse.bass_utils import run_bass_kernel_spmd

    Q = np.asarray(Q, dtype=np.float32)
    K = np.asarray(K, dtype=np.float32)
    mask = np.asarray(mask)
    w_q = np.asarray(w_q, dtype=np.float32)
    w_k = np.asarray(w_k, dtype=np.float32)
    w_v = np.asarray(w_v, dtype=np.float32)
    w_o = np.asarray(w_o, dtype=np.float32)

    wq8 = np.ascontiguousarray(_dr_pack(w_q * WS)).astype(NPF8)
    wk8 = np.ascontiguousarray(_dr_pack(w_k * WS)).astype(NPF8)
    wvb = w_v.astype(NPBF)
    wob = w_o.astype(NPBF)

    in_maps = []
    for b in range(B):
        qt = np.ascontiguousarray(Q[b].transpose(2, 1, 0))   # [DQ, M, N]
        kt = np.ascontiguousarray(K[b].transpose(2, 1, 0))   # [DK, M, KN]
        in_maps.append(
            {
                "QT8": np.ascontiguousarray(_dr_pack(qt)).astype(NPF8),
                "KT8": np.ascontiguousarray(_dr_pack(kt)).astype(NPF8),
                "KTB": kt.astype(NPBF),
                "MT": np.ascontiguousarray(mask[b].T).astype(NPBF),
                "WQ8": wq8,
                "WK8": wk8,
                "WVB": wvb,
                "WOB": wob,
            }
        )

    nc = _get_nc()
    r = run_bass_kernel_spmd(nc, in_maps, core_ids=list(range(B)), trace=False)
    return np.stack([r.results[b]["out"] for b in range(B)], axis=0)


if __name__ == "__main__":
    rng = np.random.default_rng(0)
    inputs = {
        "Q": rng.standard_normal((B, N, M, DQ), dtype=np.float32),
        "K": rng.standard_normal((B, KN, M, DK), dtype=np.float32),
        "mask": rng.integers(0, 2, (B, N, KN)).astype(np.int32),
        "w_q": (rng.standard_normal((DQ, DV), dtype=np.float32) * 0.02),
        "w_k": (rng.standard_normal((DK, DV), dtype=np.float32) * 0.02),
        "w_v": (rng.standard_normal((DK, DV), dtype=np.float32) * 0.02),
        "w_o": (rng.standard_normal((DV, DV), dtype=np.float32) * 0.02),
    }
    out = kernel(**inputs)
    print("out", out.shape, out.dtype, float(np.abs(out).max()))


# revision 22
# speedup vs baseline: 1.3845x; 1.0151x over previous
"""EquiMHA Trainium2 kernel.

Data-parallel over batch B=8 across the 8 NeuronCores (one batch element per
core, weights replicated, no collectives).

Per-core computation for batch b (N=512, M=4, KN=512, DQ=DK=512, DV=1024,
H=16, D=64):
  Qp = Q[b] @ w_q, Kp = K[b] @ w_k, Vp = K[b] @ w_v
  E[h,n,k] = sum_{m,d} Qp[n,m,h*64+d] Kp[k,m,h*64+d] / 32
  A = masked_softmax(E)        (max-subtraction skipped: |E|/32 <= ~2, and the
                                max cancels exactly up to the +eps term)
  O[n,m,h*64+d] = sum_k A[h,n,k] Vp[k,m,h*64+d]
  out = O @ w_o

Precision strategy (tolerance is 2e-2; measured pipeline error ~1e-2):
  - Q/K-side projections run in fp8e4m3 with DoubleRow perf mode (2x PE
    rate, 256-deep contraction per pass). Host pre-quantizes Q, K and
    w_q, w_k (weights pre-scaled by 64 into fp8 normal range; the 64*64
    factor is folded into the exp scale).
  - Everything downstream (E scores, A@V, O@w_o) runs in bf16, which is
    full PE rate and halves SBUF/DMA vs f32 so the whole working set
    stays on-chip (no DRAM round trips).

Layout strategy: the host pre-transposes all inputs into the exact tile
layouts the PE wants, so the kernel does zero on-chip transposes:
  - QT8/KT8 [256,2,M,N]: [c*128+p, i, m, n] with dq = c*256+i*128+p, the
    DoubleRow pairing of two 128-deep contraction chunks.
  - P1/P2 emit qpp/kpp[h] = [(s,d) 128, mp, n|k] bf16 via partition-shifted
    psum evacuation (DoubleRow outputs land on psum partitions 0-63).
  - E^T[k,n] per head is a 2-matmul bf16 contraction over mp; softmax runs
    in [k, n] orientation: exp on ACT (bf16 out), mask multiply on Pool,
    per-column sums via a bf16 ones-vector PE matmul, reciprocal on DVE,
    and a gpsimd partition_broadcast for 1/sum (no PE broadcast matmul).
  - Vp2[kc] = [128 k, (h, mp, s, d) 4096] bf16 so the O matmul stationary
    is one contiguous 128-column slice; O psum is normalized on DVE
    directly into OT[hp] = [(h%2,d), m, n] bf16 tiles, which are exactly
    the stationary operands P7 wants. Output leaves in natural [n, m, dvo]
    orientation for contiguous stores.
"""

import numpy as np
import ml_dtypes

import concourse.bacc as bacc
import concourse.mybir as mybir
import concourse.tile as tile

F32 = mybir.dt.float32
F32R = mybir.dt.float32r
F8 = mybir.dt.float8e4
BF = mybir.dt.bfloat16
AF = mybir.ActivationFunctionType
DR = mybir.MatmulPerfMode.DoubleRow

NPF8 = ml_dtypes.float8_e4m3
NPBF = ml_dtypes.bfloat16

B, N, M, KN = 8, 512, 4, 512
DQ, DK, DV, H = 512, 512, 1024, 16
D = DV // H
HP = H // 2          # head pairs (P7 contraction chunks)
KC = KN // 128       # k chunks
NC = N // 128        # n chunks
WS = 64.0            # host pre-scale for fp8 weights
SCALE = 1.0 / 32.0   # 1/sqrt(DV)
ESCALE = SCALE / (WS * WS)  # fused into exp


def build_nc():
    nc = bacc.Bacc("TRN2", target_bir_lowering=False, debug=False, num_devices=8)

    qt8_d = nc.dram_tensor("QT8", [256, 2, M, N], F8, kind="ExternalInput")
    kt8_d = nc.dram_tensor("KT8", [256, 2, M, KN], F8, kind="ExternalInput")
    ktb_d = nc.dram_tensor("KTB", [DK, M, KN], BF, kind="ExternalInput")
    mt_d = nc.dram_tensor("MT", [KN, N], BF, kind="ExternalInput")
    wq8_d = nc.dram_tensor("WQ8", [256, 2, DV], F8, kind="ExternalInput")
    wk8_d = nc.dram_tensor("WK8", [256, 2, DV], F8, kind="ExternalInput")
    wvb_d = nc.dram_tensor("WVB", [DK, DV], BF, kind="ExternalInput")
    wob_d = nc.dram_tensor("WOB", [DV, DV], BF, kind="ExternalInput")
    out_d = nc.dram_tensor("out", [N, M, DV], F32, kind="ExternalOutput")

    def ecopy(eng, dst, src):
        if eng is nc.scalar:
            nc.scalar.copy(dst, src)
        else:
            eng.tensor_copy(dst, src)

    with tile.TileContext(nc) as tc:
        with tc.tile_pool(name="persist", bufs=1) as persist:
            onesf = persist.tile([128, 128], F32, name="onesf")
            nc.vector.memset(onesf, 1.0)
            onesb = persist.tile([128, 1], BF, name="onesb")
            nc.vector.tensor_copy(onesb, onesf[:, 0:1])

            maskT2 = [persist.tile([128, 2, N], BF, name=f"mT{p}") for p in range(2)]
            vp2 = [persist.tile([128, M * DV], BF, name=f"vp2_{kc}") for kc in range(KC)]
            qpp = [persist.tile([128, 2, N], BF, name=f"qpp{h}") for h in range(H)]
            kpp = [persist.tile([128, 2, KN], BF, name=f"kpp{h}") for h in range(H)]
            ot = [persist.tile([128, M, N], BF, name=f"ot{hp}") for hp in range(HP)]
            wob = [persist.tile([128, DV], BF, name=f"wob{hp}") for hp in range(HP)]

            # ============ projections ============
            # Phase A: P2 (fp8 DR, evac-heavy) interleaved with half of P3
            # (bf16, PE-heavy); phase B: P1 interleaved with the other half.
            # Interleaving keeps the psum-evac engines (ACT/DVE) loaded
            # continuously instead of bursting past PE per phase.
            eng_ns = {"act": 0.0, "dve": 0.0}

            def pick_eng(act_cost, dve_cost):
                if eng_ns["act"] + act_cost <= eng_ns["dve"] + dve_cost:
                    eng_ns["act"] += act_cost
                    return nc.scalar
                eng_ns["dve"] += dve_cost
                return nc.vector

            with (
                tc.tile_pool(name="xk8", bufs=1) as xk8p,
                tc.tile_pool(name="w8", bufs=1) as w8p,
                tc.tile_pool(name="xkb", bufs=1) as xkbp,
                tc.tile_pool(name="wvb", bufs=1) as wvbp,
                tc.tile_pool(name="ppj", bufs=3, space="PSUM") as ppj,
                tc.tile_pool(name="ppv", bufs=2, space="PSUM") as ppv,
            ):
                xk8 = [xk8p.tile([128, 2, M, KN], F8, name=f"xk8_{c}") for c in range(2)]
                wk8 = [w8p.tile([128, 2, DV], F8, name=f"wk8_{c}") for c in range(2)]
                xkb = [xkbp.tile([128, M, KN], BF, name=f"xkb{c}") for c in range(4)]
                wvb = [wvbp.tile([128, DV], BF, name=f"wvb{c}") for c in range(4)]
                xq8 = [xk8p.tile([128, 2, M, N], F8, name=f"xq8_{c}") for c in range(2)]
                wq8 = [w8p.tile([128, 2, DV], F8, name=f"wq8_{c}") for c in range(2)]
                for c in range(2):
                    nc.sync.dma_start(wk8[c], wk8_d.ap()[c * 128 : (c + 1) * 128])
                # K chunks for s=0 (m=0,2) first so P2 starts sooner
                for m in (0, 2, 1, 3):
                    for c in range(2):
                        nc.sync.dma_start(
                            xk8[c][:, :, m, :],
                            kt8_d.ap()[c * 128 : (c + 1) * 128, :, m, :],
                        )
                for c in range(4):
                    nc.sync.dma_start(wvb[c], wvb_d.ap()[c * 128 : (c + 1) * 128])
                    nc.sync.dma_start(xkb[c], ktb_d.ap()[c * 128 : (c + 1) * 128])
                for c in range(2):
                    nc.sync.dma_start(wq8[c], wq8_d.ap()[c * 128 : (c + 1) * 128])
                    nc.sync.dma_start(xq8[c], qt8_d.ap()[c * 128 : (c + 1) * 128])
                for kc in range(KC):
                    nc.sync.dma_start(
                        maskT2[kc // 2][:, kc % 2, :],
                        mt_d.ap()[kc * 128 : (kc + 1) * 128],
                    )
                for hp in range(HP):
                    nc.sync.dma_start(wob[hp], wob_d.ap()[hp * 128 : (hp + 1) * 128])

                def proj8(h, s, w_sb, x_sb, dst):
                    pq2 = ppj.tile([64, 1024], F32, name="pq2", tag="pq2")
                    for mp in range(2):
                        m = 2 * mp + s
                        for nh in range(2):
                            for c in range(2):
                                nc.tensor.matmul(
                                    pq2[:, mp * 512 + nh * 256 : mp * 512 + (nh + 1) * 256],
                                    w_sb[c][:, :, h * 64 : (h + 1) * 64],
                                    x_sb[c][:, :, m, nh * 256 : (nh + 1) * 256],
                                    start=(c == 0),
                                    stop=(c == 1),
                                    perf_mode=DR,
                                )
                    ecopy(
                        pick_eng(1030, 1310),
                        dst[s * 64 : (s + 1) * 64, :, :],
                        pq2.rearrange("p (t n) -> p t n", t=2),
                    )

                p3_units = [
                    (mi, kc, dvh)
                    for mi in range(M)
                    for kc in range(KC)
                    for dvh in range(2)
                ]

                def p3_unit(u):
                    mi, kc, dvh = u
                    off = (mi // 2) * 128 + (mi % 2) * 64
                    pv = ppv.tile([128, 512], F32, name="pv", tag="pv")
                    for c in range(4):
                        nc.tensor.matmul(
                            pv,
                            xkb[c][:, mi, kc * 128 : (kc + 1) * 128],
                            wvb[c][:, dvh * 512 : (dvh + 1) * 512],
                            start=(c == 0),
                            stop=(c == 3),
                        )
                    v4 = vp2[kc].rearrange("p (h c) -> p h c", h=H)
                    ecopy(
                        pick_eng(610, 730),
                        v4[:, dvh * 8 : (dvh + 1) * 8, off : off + 64],
                        pv.rearrange("p (h d) -> p h d", h=8),
                    )

                p3i = 0
                # phase A: P2, s-major (the s=0 pass needs only half of K);
                # two P3 units per head in the second pass
                for h in range(H):
                    proj8(h, 0, wk8, xk8, kpp[h])
                for h in range(H):
                    proj8(h, 1, wk8, xk8, kpp[h])
                    if h >= 8:
                        p3_unit(p3_units[p3i]); p3i += 1
                        p3_unit(p3_units[p3i]); p3i += 1
                # phase B: P1 x16 heads; one P3 unit per head
                for h in range(H):
                    proj8(h, 0, wq8, xq8, qpp[h])
                    proj8(h, 1, wq8, xq8, qpp[h])
                    if p3i < len(p3_units):
                        p3_unit(p3_units[p3i]); p3i += 1
                while p3i < len(p3_units):
                    p3_unit(p3_units[p3i]); p3i += 1

            # ============ attention, per head (software-pipelined) ============
            # Iteration h emits: E(h) -> norm(h-2) -> sums/O(h-1).
            # Softmax sums run OFF the PE: bf16 tree-adds of the masked-exp
            # tiles on DVE (2-byte fast mode), a Pool partition_all_reduce
            # (whose output is already broadcast across partitions), and a
            # bf16 DVE reciprocal. Normalization lags two heads so nothing
            # on the PE ever waits for the reciprocal chain. Engine budget
            # per head ~3.4us on each of PE/ACT/DVE/Pool.
            import concourse.bass_isa as bass_isa

            with (
                tc.tile_pool(name="exp", bufs=4) as expp,
                tc.tile_pool(name="emp", bufs=8) as emp,
                tc.tile_pool(name="sump", bufs=2) as sump,
                tc.tile_pool(name="repp", bufs=3) as repp,
                tc.tile_pool(name="opop", bufs=3) as opop,
                tc.tile_pool(name="pse", bufs=4, space="PSUM") as pse,
                tc.tile_pool(name="pso", bufs=4, space="PSUM") as pso,  # 4+4 = 8
            ):

                def emit_e(h):
                    em = []
                    for kc in range(KC):
                        pe = pse.tile([128, N], F32, name="pe", tag="pe")
                        for mp in range(2):
                            nc.tensor.matmul(
                                pe,
                                kpp[h][:, mp, kc * 128 : (kc + 1) * 128],
                                qpp[h][:, mp, :],
                                start=(mp == 0),
                                stop=(mp == 1),
                            )
                        ex = expp.tile([128, N], BF, name="ex", tag="ex")
                        nc.scalar.activation(ex, pe, AF.Exp, scale=ESCALE)
                        emt = emp.tile([128, N], BF, name="em", tag="em")
                        eng = nc.gpsimd if (kc == 3 and h < 14) else nc.vector
                        eng.tensor_mul(emt, ex, maskT2[kc // 2][:, kc % 2, :])
                        em.append(emt)
                    return em

                def emit_so(h, em):
                    """bf16 sums tree + all-reduce + recip; O matmuls + evac"""
                    t0 = sump.tile([128, N], BF, name="t0", tag="t0")
                    nc.vector.tensor_add(t0, em[0], em[1])
                    t1 = sump.tile([128, N], BF, name="t1", tag="t1")
                    nc.vector.tensor_add(t1, em[2], em[3])
                    s_all = sump.tile([128, N], BF, name="s_all", tag="t0")
                    nc.vector.tensor_add(s_all, t0, t1)
                    s_red = sump.tile([128, N], BF, name="s_red", tag="t1")
                    nc.gpsimd.partition_all_reduce(
                        s_red, s_all, channels=128, reduce_op=bass_isa.ReduceOp.add
                    )
                    rep = repp.tile([128, N], BF, name="rep", tag="rep")
                    with nc.allow_low_precision(reason="softmax 1/sum"):
                        nc.vector.reciprocal(rep, s_red)

                    opo = opop.tile([128, 2, N], BF, name="opo", tag="opo")
                    for mp in range(2):
                        po = pso.tile([128, N], F32, name="po", tag="po")
                        for kc in range(KC):
                            nc.tensor.matmul(
                                po,
                                vp2[kc][:, h * 256 + mp * 128 : h * 256 + (mp + 1) * 128],
                                em[kc],
                                start=(kc == 0),
                                stop=(kc == KC - 1),
                            )
                        nc.scalar.copy(opo[:, mp, :], po)
                    return opo, rep

                def emit_norm(h, opo, rep):
                    """normalize O into OT tiles (bf16 fast muls, DVE/Pool;
                    late heads all-DVE so the P7 warm-up isn't gated on Pool)"""
                    hp, hs = h // 2, h % 2
                    for mp in range(2):
                        for s in range(2):
                            eng = nc.gpsimd if (mp == 1 and s == 1 and h < 13) else nc.vector
                            eng.tensor_mul(
                                ot[hp][hs * 64 : (hs + 1) * 64, 2 * mp + s, :],
                                opo[s * 64 : (s + 1) * 64, mp, :],
                                rep[s * 64 : (s + 1) * 64, :],
                            )

                em_q, so_q = {}, {}
                for h in range(H):
                    em_q[h] = emit_e(h)
                    if h >= 2:
                        emit_norm(h - 2, *so_q.pop(h - 2))
                    if h >= 1:
                        so_q[h - 1] = emit_so(h - 1, em_q.pop(h - 1))
                so_q[H - 1] = emit_so(H - 1, em_q.pop(H - 1))
                emit_norm(H - 2, *so_q.pop(H - 2))
                emit_norm(H - 1, *so_q.pop(H - 1))

            # ============ P7: output projection (bf16) ============
            # The first four psum groups run contraction chunks hp=0..5
            # before any hp>=6 step, so the PE keeps streaming while the
            # last two heads\' normalized OT tiles are still being written.
            with (
                tc.tile_pool(name="outst", bufs=4) as outstp,
                tc.tile_pool(name="psf", bufs=4, space="PSUM") as psf,
            ):
                units = [(mi, ncc) for mi in range(M) for ncc in range(NC)]

                def p7_mms(pf, mi, ncc, dvh, hps, first, last):
                    for hp in hps:
                        nc.tensor.matmul(
                            pf,
                            ot[hp][:, mi, ncc * 128 : (ncc + 1) * 128],
                            wob[hp][:, dvh * 512 : (dvh + 1) * 512],
                            start=(hp == first),
                            stop=(hp == last),
                        )

                def p7_finish(u, pf0, pf1):
                    mi, ncc = u
                    ost = outstp.tile([128, 2, 512], F32, name="ost", tag="ost")
                    nc.scalar.copy(ost[:, 0, :], pf0)
                    nc.vector.tensor_copy(ost[:, 1, :], pf1)
                    nc.sync.dma_start(
                        out_d.ap()[ncc * 128 : (ncc + 1) * 128, mi, :],
                        ost.rearrange("p a b -> p (a b)"),
                    )

                # warm stretch: 4 psum groups of hp0..5 for the first 2 units
                warm = []
                for u in units[:2]:
                    mi, ncc = u
                    pfs = []
                    for dvh in range(2):
                        pf = psf.tile([128, 512], F32, name="pf", tag="pf")
                        p7_mms(pf, mi, ncc, dvh, range(6), 0, HP - 1)
                        pfs.append(pf)
                    warm.append((u, pfs))
                for u, pfs in warm:
                    mi, ncc = u
                    for dvh in range(2):
                        p7_mms(pfs[dvh], mi, ncc, dvh, range(6, HP), 0, HP - 1)
                    p7_finish(u, *pfs)
                for u in units[2:-1]:
                    mi, ncc = u
                    pfs = []
                    for dvh in range(2):
                        pf = psf.tile([128, 512], F32, name="pf", tag="pf")
                        p7_mms(pf, mi, ncc, dvh, range(HP), 0, HP - 1)
                        pfs.append(pf)
                    p7_finish(u, *pfs)
                # last unit: per-dvh half evac + half DMA for a short drain
                mi, ncc = units[-1]
                for dvh in range(2):
                    pf = psf.tile([128, 512], F32, name="pf", tag="pf")
                    p7_mms(pf, mi, ncc, dvh, range(HP), 0, HP - 1)
                    osh = outstp.tile([128, 512], F32, name="osh", tag="osh")
                    if dvh == 0:
                        nc.scalar.copy(osh[0:64, :], pf[0:64, :])
                        nc.vector.tensor_copy(osh[64:128, :], pf[64:128, :])
                    else:
                        nc.vector.tensor_copy(osh[0:64, :], pf[0:64, :])
                        nc.scalar.copy(osh[64:128, :], pf[64:128, :])
                    nc.sync.dma_start(
                        out_d.ap()[
                            ncc * 128 : (ncc + 1) * 128,
                            mi,
                            dvh * 512 : (dvh + 1) * 512,
                        ],
                        osh,
                    )

    nc.compile()
    return nc


_NC_CACHE = None


def _get_nc():
    global _NC_CACHE
    if _NC_CACHE is None:
        _NC_CACHE = build_nc()
    return _NC_CACHE


def _dr_pack(a):
    """[512, ...] rows dq = c*256 + i*128 + p -> [256 = c*128+p, 2 = i, ...]"""
    s = a.shape[1:]
    return a.reshape(2, 2, 128, *s).transpose(0, 2, 1, *range(3, 3 + len(s))).reshape(
        256, 2, *s
    )


def kernel(Q, K, mask, w_q, w_k, w_v, w_o):
    from concourse.bass_utils import run_bass_kernel_spmd

    Q = np.asarray(Q, dtype=np.float32)
    K = np.asarray(K, dtype=np.float32)
    mask = np.asarray(mask)
    w_q = np.asarray(w_q, dtype=np.float32)
    w_k = np.asarray(w_k, dtype=np.float32)
    w_v = np.asarray(w_v, dtype=np.float32)
    w_o = np.asarray(w_o, dtype=np.float32)

    wq8 = np.ascontiguousarray(_dr_pack(w_q * WS)).astype(NPF8)
    wk8 = np.ascontiguousarray(_dr_pack(w_k * WS)).astype(NPF8)
    wvb = w_v.astype(NPBF)
    wob = w_o.astype(NPBF)

    in_maps = []
    for b in range(B):
        qt = np.ascontiguousarray(Q[b].transpose(2, 1, 0))   # [DQ, M, N]
        kt = np.ascontiguousarray(K[b].transpose(2, 1, 0))   # [DK, M, KN]
        in_maps.append(
            {
                "QT8": np.ascontiguousarray(_dr_pack(qt)).astype(NPF8),
                "KT8": np.ascontiguousarray(_dr_pack(kt)).astype(NPF8),
                "KTB": kt.astype(NPBF),
                "MT": np.ascontiguousarray(mask[b].T).astype(NPBF),
                "WQ8": wq8,
                "WK8": wk8,
                "WVB": wvb,
                "WOB": wob,
            }
        )

    nc = _get_nc()
    r = run_bass_kernel_spmd(nc, in_maps, core_ids=list(range(B)), trace=False)
    return np.stack([r.results[b]["out"] for b in range(B)], axis=0)


if __name__ == "__main__":
    rng = np.random.default_rng(0)
    inputs = {
        "Q": rng.standard_normal((B, N, M, DQ), dtype=np.float32),
        "K": rng.standard_normal((B, KN, M, DK), dtype=np.float32),
        "mask": rng.integers(0, 2, (B, N, KN)).astype(np.int32),
        "w_q": (rng.standard_normal((DQ, DV), dtype=np.float32) * 0.02),
        "w_k": (rng.standard_normal((DK, DV), dtype=np.float32) * 0.02),
        "w_v": (rng.standard_normal((DK, DV), dtype=np.float32) * 0.02),
        "w_o": (rng.standard_normal((DV, DV), dtype=np.float32) * 0.02),
    }
    out = kernel(**inputs)
    print("out", out.shape, out.dtype, float(np.abs(out).max()))
